# revision 17
# baseline (speedup 1.0000x reference)
"""RPN (conv3x3 -> relu -> cls/bbox heads -> objectness top-512 decode) on 8 TRN2 cores.

Data parallel: 2 images per core. Everything (conv, heads, top-k, gather, decode)
runs on-device in fp32; host only shards/pads inputs and concatenates outputs.
"""
import os
import numpy as np

import concourse.bass as bass
import concourse.bacc as bacc
import concourse.mybir as mybir
import concourse.tile as tile
from concourse import bass_utils, library_config
from concourse.masks import make_identity

# ---- problem geometry (hardcoded; must match the grader's setup_inputs) ----
BATCH = 16
N_CORES = 8
IMGS = 2                      # images per core
HF, WF, CIN, OC = 48, 80, 512, 512
KA = 9                        # anchors per position
NROI = 512
WINDOW_SIZES = [2, 4, 8]

HP, WP = HF + 2, WF + 2       # padded grid 50 x 82
NPOS_P = HP * WP              # 4100
NT_X = 33                     # pos' tiles of 128 (ceil 4100/128)
NPP = NT_X * 128              # 4224
NPOS = HF * WF                # 3840
NBAND = 8
BROWS = HF // NBAND           # 6
BN = BROWS * WF               # 480 positions per band
NANCH = NPOS * KA             # 34560
VOCAB = 50176                 # topk vocab (>50000, %128==0)
VROW = VOCAB // 16            # 3136
NEG = -1.0e30
TROW = 40                     # gather-table row floats: 36 bbox + Ax + Ay + 2 pad
WHM = 96                      # head matmul M (padded for 32-aligned PSUM groups)

FP = mybir.dt.float32
BF = mybir.dt.bfloat16
U32 = mybir.dt.uint32
I32 = mybir.dt.int32

_CACHE = {}


def _anchors_np():
    xx, yy = np.meshgrid(np.arange(WF), np.arange(HF))
    axx = xx.astype(np.float32)
    ayy = yy.astype(np.float32)
    wg, hg = np.meshgrid(WINDOW_SIZES, WINDOW_SIZES)
    ww = wg.reshape(-1).astype(np.float32)
    hh = hg.reshape(-1).astype(np.float32)
    valid = ((axx[:, :, None] - ww / 2 >= 0)
             & (axx[:, :, None] + ww / 2 <= WF)
             & (ayy[:, :, None] - hh / 2 >= 0)
             & (ayy[:, :, None] + hh / 2 <= HF))
    return axx, ayy, ww, hh, valid


def _build():
    stage = os.environ.get("KSTAGE", "full")
    key = (stage, os.environ.get("KSUB", ""))
    if ("nc", key) in _CACHE:
        return _CACHE[("nc", key)]
    sub = set(os.environ.get("KSUB", "x,conv,s,bb,table,iota,prefill,lib").split(","))
    nc = bacc.Bacc(None, target_bir_lowering=False, debug=False)

    xin_d = nc.dram_tensor("xin", [IMGS, NPP, CIN], FP, kind="ExternalInput")
    w_d = nc.dram_tensor("wt", [128, 9 * 4 * 4 * 128], FP, kind="ExternalInput")
    b1_d = nc.dram_tensor("b1t", [128, 4], FP, kind="ExternalInput")
    wh_d = nc.dram_tensor("wht", [128, 4 * WHM], FP, kind="ExternalInput")
    bh_d = nc.dram_tensor("bht", [WHM, 1], FP, kind="ExternalInput")
    mask_d = nc.dram_tensor("maskb", [KA, NPOS], BF, kind="ExternalInput")
    atab_d = nc.dram_tensor("atab", [NPOS, 2], FP, kind="ExternalInput")
    wk_d = nc.dram_tensor("wk", [128, KA], FP, kind="ExternalInput")
    hk_d = nc.dram_tensor("hk", [128, KA], FP, kind="ExternalInput")

    out_d = nc.dram_tensor("out", [IMGS, NROI, 4], FP, kind="ExternalOutput")

    # DRAM scratch (exposed as outputs in debug stages)
    dbg = stage != "full"
    kw = {"kind": "ExternalOutput"} if dbg else {}
    sflat_d = nc.dram_tensor("sflat", [IMGS * VOCAB], FP, **kw)
    table_d = nc.dram_tensor("table", [IMGS * NPOS, TROW], FP, **kw)
    idxflat_d = nc.dram_tensor("idxflat", [IMGS, 2 * NROI // 2], U32, **kw)

    with tile.TileContext(nc) as tc:
        sb = tc.alloc_tile_pool(name="sb", bufs=1)
        xtp = tc.alloc_tile_pool(name="xtp", bufs=3)
        hbp = tc.alloc_tile_pool(name="hbp", bufs=1)
        smp = tc.alloc_tile_pool(name="smp", bufs=3)
        ps_tp = tc.alloc_tile_pool(name="ps_tp", bufs=2, space="PSUM")
        ps_cv = tc.alloc_tile_pool(name="ps_cv", bufs=int(os.environ.get("KCVBUFS", "3")), space="PSUM")
        ps_hd = tc.alloc_tile_pool(name="ps_hd", bufs=2, space="PSUM")
        ps_bt = tc.alloc_tile_pool(name="ps_bt", bufs=1, space="PSUM")

        if "lib" in sub:
            nc.gpsimd.load_library(library_config.topk)

        # ------- constants -------
        ident = sb.tile([128, 128], FP)
        make_identity(nc, ident[:])
        w_sb = sb.tile([128, 9 * 4 * 4 * 128], FP)
        nc.sync.dma_start(w_sb[:], w_d[:])
        b1_sb = sb.tile([128, 4], FP)
        nc.sync.dma_start(b1_sb[:], b1_d[:])
        wh_sb = sb.tile([128, 4 * WHM], FP)
        nc.sync.dma_start(wh_sb[:], wh_d[:])
        bh_sb = sb.tile([WHM, 1], FP)
        nc.sync.dma_start(bh_sb[:], bh_d[:])
        mask_sb = sb.tile([73, NPOS], BF)
        nc.sync.dma_start(mask_sb[64:73, :], mask_d[:])
        wk_sb = sb.tile([128, KA], FP)
        nc.sync.dma_start(wk_sb[:], wk_d[:])
        hk_sb = sb.tile([128, KA], FP)
        nc.sync.dma_start(hk_sb[:], hk_d[:])
        iota9 = sb.tile([128, 4, KA], FP)
        if "iota" in sub:
            nc.gpsimd.iota(iota9[:], pattern=[[0, 4], [1, KA]], base=0,
                           channel_multiplier=0,
                           allow_small_or_imprecise_dtypes=True)

        negc = sb.tile([128, VOCAB // 128], FP)   # [128, 392]
        nc.vector.memset(negc[:], NEG)

        # prefill sflat with NEG (both images)
        for img in range(IMGS if "prefill" in sub else 0):
            nc.sync.dma_start(
                sflat_d[:].rearrange("(i p c) -> i p c", i=IMGS, p=128)[img],
                negc[:])

        # A-columns of the gather tables (once per image buffer)
        acol = sb.tile([128, 30, 2], FP)
        if "table" in sub:
            nc.sync.dma_start(
                acol[:],
                atab_d[:].rearrange("(t p) j -> p t j", p=128))
            for img in range(IMGS):
                nc.sync.dma_start(
                    table_d[:].rearrange("(i t p) j -> i p t j", i=IMGS, p=128)[img, :, :, 36:38],
                    acol[:])

        # topk in/out must be raw SBUF tensors
        tkin = nc.alloc_sbuf_tensor("tkin", [32, VROW], FP)
        tkout1 = nc.alloc_sbuf_tensor("tkout1", [32, 32], U32)
        tkout2 = nc.alloc_sbuf_tensor("tkout2", [32, 32], U32)

        x_pad = sb.tile([128, 4 * NPP], FP)
        bb_sb = sb.tile([36, NPOS], FP)

        # ================= per-image conv + heads =================
        for img in range(IMGS if "x" in sub else 0):
            # ---- load + transpose x to ch-major padded ----
            for t in range(NT_X):
                xt = xtp.tile([128, CIN], FP, tag="xt")
                nc.sync.dma_start(xt[:], xin_d[img, t * 128:(t + 1) * 128, :])
                for ct in range(4):
                    tp_ps = ps_tp.tile([128, 128], FP, tag="tp")
                    nc.tensor.transpose(tp_ps[:], xt[:, ct * 128:(ct + 1) * 128], ident[:])
                    nc.vector.tensor_copy(
                        x_pad[:, ct * NPP + t * 128: ct * NPP + (t + 1) * 128],
                        tp_ps[:])

            # ---- conv + heads per band ----
            for band in range(NBAND if "conv" in sub else 0):
                hband = hbp.tile([128, 4, BN], FP, tag="hband")
                for oct_ in range(4):
                    hp = ps_cv.tile([128, BN], FP, tag="cv")
                    first = True
                    for g in range(9):
                        dy, dx = g // 3, g % 3
                        base_row = band * BROWS + dy
                        for ct in range(4):
                            off = ct * NPP + base_row * WP + dx
                            rhs = x_pad[:, off: off + BROWS * WP] \
                                .rearrange("p (r c) -> p r c", r=BROWS)[:, :, :WF]
                            widx = ((g * 4 + ct) * 4 + oct_) * 128
                            nc.tensor.matmul(hp[:], w_sb[:, widx: widx + 128], rhs,
                                             start=first, stop=(g == 8 and ct == 3))
                            first = False
                    nc.scalar.activation(hband[:, oct_, :], hp[:],
                                         mybir.ActivationFunctionType.Relu,
                                         bias=b1_sb[:, oct_:oct_ + 1])
                hd = ps_hd.tile([WHM, BN], FP, tag="hd")
                for ct in range(4):
                    nc.tensor.matmul(hd[:], wh_sb[:, ct * WHM:(ct + 1) * WHM],
                                     hband[:, ct, :], start=(ct == 0), stop=(ct == 3))
                # s rows (64..72): add head bias + validity mask, straight to DRAM
                sband = smp.tile([73, BN], FP, tag="sband")
                nc.vector.scalar_tensor_tensor(
                    sband[64:73, :], hd[64:73, :], bh_sb[64:73, :],
                    mask_sb[64:73, band * BN:(band + 1) * BN],
                    op0=mybir.AluOpType.add, op1=mybir.AluOpType.add)
                if "s" in sub:
                    nc.sync.dma_start(
                        bass.AP(sflat_d, img * VOCAB + band * BN, [[NPOS, KA], [1, BN]]),
                        sband[64:73, :])
                # bbox rows 0..35: bias add (exp for tw/th happens post-transpose)
                nc.vector.tensor_scalar_add(bb_sb[0:36, band * BN:(band + 1) * BN],
                                            hd[0:36, :], bh_sb[0:36, :])

            # ---- write gather table: transpose bb [36, 3840] -> [3840, 36] ----
            for t in range(30 if "bb" in sub else 0):
                bt = ps_bt.tile([128, 36], FP, tag="bt")
                nc.tensor.transpose(bt[:], bb_sb[:, t * 128:(t + 1) * 128],
                                    ident[0:36, 0:36])
                bts = smp.tile([128, 36], FP, tag="bts")
                nc.vector.tensor_copy(bts[:, 0:18], bt[:, 0:18])
                nc.scalar.activation(bts[:, 18:36], bt[:, 18:36],
                                     mybir.ActivationFunctionType.Exp)
                nc.sync.dma_start(
                    table_d[img * NPOS + t * 128: img * NPOS + (t + 1) * 128, 0:36],
                    bts[:])

        # ================= top-k phase (both images) =================
        run_topk = stage in ("topk", "full")
        run_decode = stage == "full"
        if not run_decode:
            # debug: still produce "out" so the output exists
            zz = smp.tile([128, 16], FP, tag="zz")
            nc.vector.memset(zz[:], 0.0)
            for img in range(IMGS):
                nc.sync.dma_start(
                    bass.AP(out_d, img * NROI * 4, [[16, 128], [4, 4], [1, 4]]), zz[:])
        tkin_ap = tkin[:]
        sflat_ap = sflat_d[:].rearrange("(r v) -> r v", r=32)
        if run_topk:
            nc.sync.dma_start(tkin_ap, sflat_ap)
            nc.gpsimd.topk(tkout1[:], tkin[:], tokens=IMGS, vocab_size=VOCAB, k=256)

        negc1 = negc[:, 0:1]
        for img in range(IMGS if run_topk else 0):
            # stash round-1 indices (ascending rank a: idxflat[img, a])
            nc.sync.dma_start(
                idxflat_d[img, 0:256].rearrange("(r c) -> r c", r=16),
                tkout1[img * 16:(img + 1) * 16, 16:32])
            # scatter NEG into sflat at the 256 found positions (by index)
            idx1 = smp.tile([128, 2], U32, tag="idx1")
            nc.sync.dma_start(
                idx1[:],
                bass.AP(idxflat_d, img * 512, [[1, 128], [128, 2]]))
            for j in range(2):
                nc.gpsimd.indirect_dma_start(
                    out=sflat_d[:].rearrange("(n o) -> n o", o=1),
                    out_offset=bass.IndirectOffsetOnAxis(ap=idx1[:, j:j + 1], axis=0),
                    in_=negc1,
                    in_offset=None,
                    element_offset=img * VOCAB)

        if run_topk:
            nc.sync.dma_start(tkin_ap, sflat_ap)
            nc.gpsimd.topk(tkout2[:], tkin[:], tokens=IMGS, vocab_size=VOCAB, k=256)
            for img in range(IMGS):
                nc.sync.dma_start(
                    idxflat_d[img, 256:512].rearrange("(r c) -> r c", r=16),
                    tkout2[img * 16:(img + 1) * 16, 16:32])

        # ================= gather + decode per image =================
        for img in range(IMGS if run_decode else 0):
            # idx in output order: rank = 4*p + c
            idxrev = smp.tile([1, 512], U32, tag="idxrev")
            nc.sync.dma_start(
                idxrev[:],
                bass.AP(idxflat_d, img * 512 + 255, [[1, 1], [256, 2], [-1, 256]]))
            idxr = smp.tile([128, 4], U32, tag="idxr")
            nc.sync.dma_start(idxr[:], idxrev[:])
            idxf = smp.tile([128, 4], FP, tag="idxf")
            nc.vector.tensor_copy(idxf[:], idxr[:])
            # k = number of thresholds passed; pos = e - 3840k
            kf = smp.tile([128, 4], FP, tag="kf")
            nc.vector.memset(kf[:], 0.0)
            for j in range(1, KA):
                nc.vector.scalar_tensor_tensor(
                    kf[:], idxf[:], float(j * NPOS), kf[:],
                    op0=mybir.AluOpType.is_ge, op1=mybir.AluOpType.add)
            posf = smp.tile([128, 4], FP, tag="posf")
            nc.vector.scalar_tensor_tensor(
                posf[:], kf[:], float(-NPOS), idxf[:],
                op0=mybir.AluOpType.mult, op1=mybir.AluOpType.add)
            posi = smp.tile([128, 4], I32, tag="posi")
            nc.vector.tensor_copy(posi[:], posf[:])

            gath = smp.tile([128, 4, TROW], FP, tag="gath")
            for c in range(4):
                nc.gpsimd.indirect_dma_start(
                    out=gath[:, c, :],
                    out_offset=None,
                    in_=table_d[:],
                    in_offset=bass.IndirectOffsetOnAxis(ap=posi[:, c:c + 1], axis=0),
                    element_offset=img * NPOS * TROW)

            # one-hot select over k
            oh = smp.tile([128, 4, KA], FP, tag="oh")
            nc.vector.tensor_tensor(
                out=oh[:], in0=kf[:].unsqueeze(2).to_broadcast([128, 4, KA]),
                in1=iota9[:], op=mybir.AluOpType.is_equal)

            def sel(dst, src3):
                tmp = smp.tile([128, 4, KA], FP, tag="seltmp")
                nc.vector.tensor_tensor(out=tmp[:], in0=oh[:], in1=src3,
                                        op=mybir.AluOpType.mult)
                nc.vector.tensor_reduce(out=dst, in_=tmp[:],
                                        axis=mybir.AxisListType.X,
                                        op=mybir.AluOpType.add)

            tx = smp.tile([128, 4], FP, tag="tx"); sel(tx[:], gath[:, :, 0:KA])
            ty = smp.tile([128, 4], FP, tag="ty"); sel(ty[:], gath[:, :, KA:2 * KA])
            tw = smp.tile([128, 4], FP, tag="tw"); sel(tw[:], gath[:, :, 2 * KA:3 * KA])
            th = smp.tile([128, 4], FP, tag="th"); sel(th[:], gath[:, :, 3 * KA:4 * KA])
            wsel = smp.tile([128, 4], FP, tag="wsel")
            sel(wsel[:], wk_sb[:].unsqueeze(1).to_broadcast([128, 4, KA]))
            hsel = smp.tile([128, 4], FP, tag="hsel")
            sel(hsel[:], hk_sb[:].unsqueeze(1).to_broadcast([128, 4, KA]))

            ob = smp.tile([128, 4, 4], FP, tag="ob")
            # xx = Ax - tx*ww ; yy = Ay - ty*hh ; wwd = ww*tw ; hhd = hh*th
            t1 = smp.tile([128, 4], FP, tag="t1")
            nc.vector.tensor_tensor(out=t1[:], in0=tx[:], in1=wsel[:], op=mybir.AluOpType.mult)
            nc.vector.tensor_tensor(out=ob[:, :, 0], in0=gath[:, :, 36], in1=t1[:],
                                    op=mybir.AluOpType.subtract)
            t2 = smp.tile([128, 4], FP, tag="t2")
            nc.vector.tensor_tensor(out=t2[:], in0=ty[:], in1=hsel[:], op=mybir.AluOpType.mult)
            nc.vector.tensor_tensor(out=ob[:, :, 1], in0=gath[:, :, 37], in1=t2[:],
                                    op=mybir.AluOpType.subtract)
            nc.vector.tensor_tensor(out=ob[:, :, 2], in0=wsel[:], in1=tw[:], op=mybir.AluOpType.mult)
            nc.vector.tensor_tensor(out=ob[:, :, 3], in0=hsel[:], in1=th[:], op=mybir.AluOpType.mult)

            # out rows: rank = 4*p + c
            nc.sync.dma_start(
                bass.AP(out_d, img * NROI * 4, [[16, 128], [4, 4], [1, 4]]),
                ob[:])

        for p in [ps_bt, ps_hd, ps_cv, ps_tp, smp, hbp, xtp, sb]:
            p.release()

    nc.finalize()
    _CACHE[("nc", key)] = nc
    return nc


def _host_prep(features, conv1_w, conv1_b, cls_w, cls_b, bbox_w, bbox_b):
    axx, ayy, ww, hh, valid = _anchors_np()

    # conv weights -> [128, ((g*4+ct)*4+oct)*128 + oc]
    w = np.ascontiguousarray(
        conv1_w.reshape(3, 3, 4, 128, 4, 128).transpose(3, 0, 1, 2, 4, 5)
    ).reshape(128, -1)
    b1t = np.ascontiguousarray(conv1_b.reshape(4, 128).T)  # [128, 4]

    cw = cls_w[0, 0]     # [512, 18]
    cb = cls_b
    bw = bbox_w[0, 0]    # [512, 36]
    bb = bbox_b
    whead = np.zeros((CIN, 96), dtype=np.float32)
    whead[:, 0:36] = bw
    whead[:, 64:73] = cw[:, KA:] - cw[:, :KA]
    wht = np.ascontiguousarray(
        whead.reshape(4, 128, 96).transpose(1, 0, 2)).reshape(128, 4 * 96)
    bht = np.zeros((96, 1), dtype=np.float32)
    bht[0:36, 0] = bb
    bht[64:73, 0] = cb[KA:] - cb[:KA]

    import ml_dtypes
    maskb = np.where(valid.reshape(NPOS, KA).T, np.float32(0.0),
                     np.float32(NEG)).astype(ml_dtypes.bfloat16)

    atab = np.stack([np.tile(axx.reshape(-1), 1), np.tile(ayy.reshape(-1), 1)],
                    axis=1).astype(np.float32)          # [3840, 2]
    wk = np.tile(ww[None, :], (128, 1)).astype(np.float32)
    hk = np.tile(hh[None, :], (128, 1)).astype(np.float32)

    common = {"wt": w.astype(np.float32), "b1t": b1t.astype(np.float32),
              "wht": wht.astype(np.float32), "bht": bht,
              "maskb": maskb, "atab": atab, "wk": wk, "hk": hk}

    in_maps = []
    for c in range(N_CORES):
        sh = features[c * IMGS:(c + 1) * IMGS]           # [2, 48, 80, 512]
        xp = np.zeros((IMGS, HP, WP, CIN), dtype=np.float32)
        xp[:, 1:HF + 1, 1:WF + 1, :] = sh
        xp = xp.reshape(IMGS, NPOS_P, CIN)
        xin = np.zeros((IMGS, NPP, CIN), dtype=np.float32)
        xin[:, :NPOS_P, :] = xp
        in_maps.append({"xin": xin, **common})
    return in_maps


def kernel(features, conv1_w, conv1_b, cls_w, cls_b, bbox_w, bbox_b):
    features = np.asarray(features, dtype=np.float32)
    conv1_w = np.asarray(conv1_w, dtype=np.float32)
    conv1_b = np.asarray(conv1_b, dtype=np.float32)
    cls_w = np.asarray(cls_w, dtype=np.float32)
    cls_b = np.asarray(cls_b, dtype=np.float32)
    bbox_w = np.asarray(bbox_w, dtype=np.float32)
    bbox_b = np.asarray(bbox_b, dtype=np.float32)

    nc = _build()
    in_maps = _host_prep(features, conv1_w, conv1_b, cls_w, cls_b, bbox_w, bbox_b)
    res = bass_utils.run_bass_kernel_spmd(nc, in_maps, core_ids=list(range(N_CORES)))
    _CACHE["last_results"] = res.results
    out = np.concatenate([res.results[c]["out"] for c in range(N_CORES)], axis=0)
    return out.astype(np.float32)
